# revision 8
# baseline (speedup 1.0000x reference)
"""EnhancedGraphSAGE forward on 8 Trainium2 NeuronCores (Bass/Tile).

Strategy (see spec sharding_hint):
- Nodes sharded by id across 8 cores (12500 each). Edges partitioned by
  dst-node owner; host bin-packs each core's dst nodes into windows of
  <=127 dsts whose incoming edges are grouped by source "quarter"
  (quarter = 2 cores' node range, int16-addressable in the gathered table)
  with a uniform 4 x 512 edge-slot structure per window.
- Layer 1 messages (x[src]) are host-prelaid as a sequential DMA stream
  (graph layout preprocessing only; no arithmetic on features).
- Layers 2/3 + decode gather rows with the custom dma_gather ucode
  instruction on 4 SWDGE queues (int16 indices per quarter sub-table).
- Aggregation: PE matmul msg^T @ S01 where S01 is a one-hot selection
  matrix built on DVE via iota-compare with per-slot dst rows.
- Mean division folded in as a per-dst-row reciprocal-degree scale;
  BatchNorm folded into weights on host; bias via rank-1 matmul.
- Inter-layer exchange: AllGather of each core's produced h window rows
  (bf16) into a replicated table; z exchanged f32 for the decode gathers.
"""

import os
import sys
import types
import contextlib
import ctypes

import numpy as np
import ml_dtypes

NCORES = 8
P = 128
WIN_EDGES = 2048          # 4 quarters x 512 slots
QSLOTS = 512              # slots per (window, quarter)
MAXDST = 127              # dst rows per window (row 127 = trash)
BN_EPS = 1e-5

bf16 = ml_dtypes.bfloat16

_SO_PATH = "/opt/axon/libaxon_pjrt.so"
LAST_EXEC_NS = None


def _install_ntff_shim():
    try:
        import antenv.axon_hooks  # noqa: F401
        return
    except ImportError:
        pass
    try:
        import antenv
    except ImportError:
        return
    lib = ctypes.CDLL(_SO_PATH)
    if not hasattr(lib, "axon_start_nrt_profile"):
        return
    lib.axon_start_nrt_profile.argtypes = [ctypes.POINTER(ctypes.c_int64), ctypes.c_size_t]
    lib.axon_start_nrt_profile.restype = ctypes.c_int64
    lib.axon_stop_nrt_profile.argtypes = [ctypes.c_char_p]
    lib.axon_stop_nrt_profile.restype = ctypes.c_int64

    @contextlib.contextmanager
    def _hook(output_dir, device_ids):
        import jax
        jax.devices()
        if device_ids:
            ids = (ctypes.c_int64 * len(device_ids))(*device_ids)
            rc = lib.axon_start_nrt_profile(ids, len(device_ids))
        else:
            rc = lib.axon_start_nrt_profile(None, 0)
        if rc != 0:
            raise RuntimeError(f"axon_start_nrt_profile rc={rc}")
        try:
            yield
        finally:
            lib.axon_stop_nrt_profile(str(output_dir).encode())

    mod = types.ModuleType("antenv.axon_hooks")
    _state = {"hook": _hook}
    mod.set_axon_ntff_profile_hook = lambda h: _state.__setitem__("hook", h)
    mod.get_axon_ntff_profile_hook = lambda: _state["hook"]
    sys.modules["antenv.axon_hooks"] = mod
    antenv.axon_hooks = mod


def _axon_reset():
    try:
        lib = ctypes.CDLL(_SO_PATH)
        lib.axon_reset.restype = ctypes.c_int64
        lib.axon_reset()
    except Exception:
        pass


# ---------------------------------------------------------------- preprocess

def _pack_windows(deg4, shard):
    """Bin-pack `shard` dsts (per-quarter in-degree rows deg4 [n,4]) into
    windows of <=MAXDST dsts with per-quarter edge sums <=QSLOTS.
    Returns win_of [n], row_of [n], n_windows."""
    n = deg4.shape[0]
    order = np.argsort(-deg4.sum(1), kind="stable")
    cap = max(n // 64, 8)
    wq = np.zeros((cap, 4), np.int64)
    wn = np.zeros(cap, np.int64)
    nwin = 0
    win_of = np.zeros(n, np.int32)
    row_of = np.zeros(n, np.int32)
    for i in order:
        d4 = deg4[i]
        ok = (wn[:nwin] < MAXDST) & np.all(wq[:nwin] + d4 <= QSLOTS, axis=1)
        j = int(np.argmax(ok)) if ok.any() else -1
        if j < 0 or not ok[j]:
            j = nwin
            nwin += 1
            if nwin > cap:
                raise RuntimeError("window cap exceeded")
        win_of[i] = j
        row_of[i] = wn[j]
        wn[j] += 1
        wq[j] += d4
    return win_of, row_of, nwin


def preprocess(inputs):
    x = np.asarray(inputs["x"], np.float32)
    ei = np.asarray(inputs["edge_index"], np.int64)
    pos_ei = np.asarray(inputs["pos_edge_index"], np.int64)
    neg_ei = np.asarray(inputs["neg_edge_index"], np.int64)
    N, C = x.shape
    E = ei.shape[1]
    assert N % NCORES == 0
    SHARD = N // NCORES
    src, dst = ei[0].astype(np.int64), ei[1].astype(np.int64)

    deg = np.bincount(dst, minlength=N).astype(np.float32)
    rdeg = 1.0 / np.maximum(deg, 1.0)

    quarter = (src // (2 * SHARD)).astype(np.int64)      # quarter of each edge's src
    dcore = dst // SHARD

    # per-core window packing
    win_of = np.zeros(N, np.int32)
    row_of = np.zeros(N, np.int32)
    Ws = []
    for c in range(NCORES):
        nodes = np.arange(c * SHARD, (c + 1) * SHARD)
        # per-quarter in-degree of each dst in this shard
        m = dcore == c
        d_loc = dst[m] - c * SHARD
        q_loc = quarter[m]
        deg4 = np.zeros((SHARD, 4), np.int64)
        np.add.at(deg4, (d_loc, q_loc), 1)
        w_c, r_c, nw = _pack_windows(deg4, SHARD)
        win_of[nodes] = w_c
        row_of[nodes] = r_c
        Ws.append(nw)
    W = max(Ws)
    assert W <= 127, f"W={W} overflows int16 quarter addressing"
    QR = 2 * W * P                                        # rows per quarter table
    R = NCORES * W * P                                    # total table rows

    # global permuted position of every node
    gpos = (dst_core_pos := (np.arange(N) // SHARD) * (W * P)) + win_of * P + row_of
    del dst_core_pos

    # fill edge slots: per core, per window, per quarter
    # slot index within core: w*2048 + q*512 + k
    srcnode = np.full((NCORES, W, 4, QSLOTS), -1, np.int64)
    rowid = np.full((NCORES, W, 4, QSLOTS), MAXDST, np.int16)
    order = np.lexsort((quarter, win_of[dst], dcore))
    s_s, d_s, q_s, c_s = src[order], dst[order], quarter[order], dcore[order]
    w_s = win_of[d_s]
    r_s = row_of[d_s]
    # group boundaries over (core, window, quarter)
    key = (c_s * W + w_s) * 4 + q_s
    uniq, start = np.unique(key, return_index=True)
    counts = np.diff(np.append(start, len(key)))
    if counts.max() > QSLOTS:
        raise RuntimeError("quarter overflow: packing violated")
    k_in_grp = np.arange(len(key)) - np.repeat(start, counts)
    srcnode[c_s, w_s, q_s, k_in_grp] = s_s
    rowid[c_s, w_s, q_s, k_in_grp] = r_s.astype(np.int16)

    # int16 gather indices (local row within src quarter); pads -> 0
    idx_local = np.where(srcnode >= 0, gpos[np.clip(srcnode, 0, None)] - quarter_of_slot(srcnode, SHARD) * QR, 0)

    per_core = []
    x_bf = x.astype(bf16)
    for c in range(NCORES):
        d = {}
        # IDX [128, W*4*32] int16, wrapped 16 + replicated x8
        a = idx_local[c].reshape(W * 4, 32, 16).transpose(2, 0, 1).reshape(16, W * 4 * 32)
        d["IDX"] = np.tile(a.astype(np.int16), (8, 1))
        # ROWID bf16 [128, W*16]
        rw = rowid[c].reshape(W, 16, 128).transpose(2, 0, 1).reshape(128, W * 16)
        d["ROWID"] = rw.astype(np.float32).astype(bf16)
        # RDEG f32 [128, W]
        nodes = np.full((W, P), -1, np.int64)
        wn_c = win_of[c * SHARD:(c + 1) * SHARD]
        rn_c = row_of[c * SHARD:(c + 1) * SHARD]
        nodes[wn_c, rn_c] = np.arange(c * SHARD, (c + 1) * SHARD)
        d["nodes"] = nodes
        rd = np.ones((W, P), np.float32)
        valid = nodes >= 0
        rd[valid] = rdeg[nodes[valid]]
        d["RDEG"] = rd.T.copy()                           # [128, W]
        # XT bf16 [128, W*128]: [cc, w*128+r] = x[node(w,r), cc]
        xw = np.zeros((W, P, C), np.float32)
        xw[valid] = x[nodes[valid]]
        d["XT"] = xw.transpose(2, 0, 1).reshape(C, W * P).astype(bf16)
        # XRES bf16 [128, W*128]: [p, w*128+cc] = x[node(w,p), cc]
        d["XRES"] = xw.transpose(1, 0, 2).reshape(P, W * C).astype(bf16)
        # XSLOTS bf16 [128, W*16*128]: L1 pre-laid messages (0 for pads)
        sn = srcnode[c].reshape(W, WIN_EDGES)             # [W, 2048]
        xs = np.zeros((W, WIN_EDGES, C), bf16)
        vv = sn >= 0
        xs[vv] = x_bf[sn[vv]]
        d["XSLOTS"] = xs.reshape(W, 16, 128, C).transpose(2, 0, 1, 3).reshape(128, W * 16 * C)
        per_core.append(d)

    # ------------------------------------------------ decode edge grouping
    de = np.concatenate([pos_ei, neg_ei], axis=1)         # [2, 400K]
    ND = de.shape[1]
    assert ND % NCORES == 0
    DSH = ND // NCORES
    gs_all = gpos[de[0]]
    gd_all = gpos[de[1]]
    qs_all = de[0] // (2 * SHARD)
    qd_all = de[1] // (2 * SHARD)

    # per core, group by (qs, qd); uniform tiles per group across cores
    grp_edges = []                                        # [core][16] -> edge ids
    gt = np.zeros((NCORES, 16), np.int64)
    for c in range(NCORES):
        eids = np.arange(c * DSH, (c + 1) * DSH)
        gkey = qs_all[eids] * 4 + qd_all[eids]
        lists = [eids[gkey == g] for g in range(16)]
        grp_edges.append(lists)
        gt[c] = [(len(l) + 127) // 128 for l in lists]
    gtiles = gt.max(axis=0)                               # tiles per group
    # round call chunks: calls of up to 8 tiles
    call_plan = []                                        # (group, ntiles) per call
    for g in range(16):
        t = int(gtiles[g])
        while t > 0:
            k = min(8, t)
            call_plan.append((g, k))
            t -= k
    TD = int(gtiles.sum())                                # total decode tiles
    NCALLS_D = len(call_plan)
    IDXCOLS_D = sum(k * 8 for _, k in call_plan)

    for c in range(NCORES):
        dis = np.zeros((16, IDXCOLS_D), np.int16)
        did = np.zeros((16, IDXCOLS_D), np.int16)
        slotmap = np.full((TD, 128), -1, np.int64)
        col = 0
        tcursor = {g: 0 for g in range(16)}
        gstart = np.concatenate([[0], np.cumsum(gtiles)])[:16]
        for (g, k) in call_plan:
            t0 = tcursor[g]
            eids = grp_edges[c][g]
            sl = np.full(k * 128, -1, np.int64)
            lo = t0 * 128
            take = eids[lo:lo + k * 128]
            sl[:len(take)] = take
            # record slot map at global tile positions
            gtile0 = int(gstart[g]) + t0
            slotmap[gtile0:gtile0 + k].reshape(-1)[:] = sl
            s_loc = np.where(sl >= 0, gs_all[np.clip(sl, 0, None)] - (g // 4) * QR, 0).astype(np.int16)
            d_loc = np.where(sl >= 0, gd_all[np.clip(sl, 0, None)] - (g % 4) * QR, 0).astype(np.int16)
            dis[:, col:col + k * 8] = s_loc.reshape(k * 8, 16).T
            did[:, col:col + k * 8] = d_loc.reshape(k * 8, 16).T
            tcursor[g] += k
            col += k * 8
        per_core[c]["DIDXS"] = np.tile(dis, (8, 1))
        per_core[c]["DIDXD"] = np.tile(did, (8, 1))
        per_core[c]["slotmap"] = slotmap

    # ------------------------------------------------ weights (BN folded)
    f32 = np.float32
    gs1 = np.asarray(inputs["g1"], f32) / np.sqrt(1.0 + BN_EPS)
    gs2 = np.asarray(inputs["g2"], f32) / np.sqrt(1.0 + BN_EPS)
    wts = {
        "WL1": (np.asarray(inputs["Wl1"], f32) * gs1).astype(bf16),
        "WR1": (np.asarray(inputs["Wr1"], f32) * gs1).astype(bf16),
        "B1": (np.asarray(inputs["bl1"], f32) * gs1 + np.asarray(inputs["b1"], f32)).astype(bf16)[None, :],
        "WL2": (np.asarray(inputs["Wl2"], f32) * gs2).astype(bf16),
        "WR2": (np.asarray(inputs["Wr2"], f32) * gs2).astype(bf16),
        "B2": (np.asarray(inputs["bl2"], f32) * gs2 + np.asarray(inputs["b2"], f32)).astype(bf16)[None, :],
        "WL3": np.asarray(inputs["Wl3"], f32).astype(bf16),
        "WR3": np.asarray(inputs["Wr3"], f32).astype(bf16),
        "B3": np.asarray(inputs["bl3"], f32).astype(bf16)[None, :],
        "WE1A": np.asarray(inputs["We1"], f32)[:64].astype(bf16),
        "WE1B": np.asarray(inputs["We1"], f32)[64:].astype(bf16),
        "BE1": np.asarray(inputs["be1"], f32)[:, None],   # [128,1] f32
        "WE2": np.asarray(inputs["We2"], f32).astype(bf16),  # [128,1]
        "BE2": float(np.asarray(inputs["be2"], f32)[0]),
    }

    meta = dict(N=N, C=C, E=E, SHARD=SHARD, W=W, QR=QR, R=R, ND=ND, DSH=DSH,
                TD=TD, NCALLS_D=NCALLS_D, IDXCOLS_D=IDXCOLS_D,
                call_plan=call_plan, OUT=64)
    return per_core, wts, meta


def quarter_of_slot(srcnode, shard):
    return np.clip(srcnode, 0, None) // (2 * shard)


# ---------------------------------------------------------------- builder

def build_program(meta):
    import concourse.bass as bass
    import concourse.bacc as bacc
    import concourse.mybir as mybir
    import concourse.tile as tile
    from concourse.masks import make_identity

    W, QR, R, C, OUT = meta["W"], meta["QR"], meta["R"], meta["C"], meta["OUT"]
    TD, call_plan, IDXCOLS_D = meta["TD"], meta["call_plan"], meta["IDXCOLS_D"]
    f32, b16, i16 = mybir.dt.float32, mybir.dt.bfloat16, mybir.dt.int16
    AF = mybir.ActivationFunctionType

    nc = bacc.Bacc("TRN2", target_bir_lowering=False, debug=False,
                   num_devices=NCORES, num_swdge_queues=4)
    qctr = [0]

    def next_q():
        q = qctr[0] % 4
        qctr[0] += 1
        return q

    def din(name, shape, dt):
        return nc.dram_tensor(name, shape, dt, kind="ExternalInput")

    IDX = din("IDX", [P, W * 4 * 32], i16)
    ROWID = din("ROWID", [P, W * 16], b16)
    RDEG = din("RDEG", [P, W], f32)
    XT = din("XT", [P, W * P], b16)
    XRES = din("XRES", [P, W * P], b16)
    XSLOTS = din("XSLOTS", [P, W * 16 * C], b16)
    DIDXS = din("DIDXS", [P, IDXCOLS_D], i16)
    DIDXD = din("DIDXD", [P, IDXCOLS_D], i16)
    WL = [din("WL1", [C, C], b16), din("WL2", [C, C], b16), din("WL3", [C, OUT], b16)]
    WR = [din("WR1", [C, C], b16), din("WR2", [C, C], b16), din("WR3", [C, OUT], b16)]
    BV = [din("B1", [1, C], b16), din("B2", [1, C], b16), din("B3", [1, OUT], b16)]
    WE1A = din("WE1A", [OUT, C], b16)
    WE1B = din("WE1B", [OUT, C], b16)
    BE1 = din("BE1", [C, 1], f32)
    WE2 = din("WE2", [C, 1], b16)
    BE2 = meta["BE2"]

    Z = nc.dram_tensor("Z", [W * P, OUT], f32, kind="ExternalOutput")
    DOUT = nc.dram_tensor("DOUT", [P, TD], f32, kind="ExternalOutput")

    bounce = [nc.dram_tensor(f"bounce{l}", [W * P, C if l < 2 else OUT],
                             b16 if l < 2 else f32) for l in range(3)]
    htab = [nc.dram_tensor(f"htab{l}", [R, C if l < 2 else OUT],
                           b16 if l < 2 else f32, addr_space="Shared") for l in range(3)]

    with tile.TileContext(nc) as tc:
        with (
            tc.tile_pool(name="persist", bufs=1) as pp,
            tc.tile_pool(name="wpool", bufs=1) as wp,
        ):
            # persistent state
            iota = pp.tile([P, 4, P], b16)
            nc.gpsimd.iota(iota[:], pattern=[[0, 4], [1, P]], base=0,
                           channel_multiplier=0, allow_small_or_imprecise_dtypes=True)
            ident = pp.tile([P, P], b16)
            make_identity(nc, ident[:])
            identf = pp.tile([P, P], f32)
            make_identity(nc, identf[:])
            ones1 = pp.tile([1, P], b16)
            nc.vector.memset(ones1[:], 1.0)

            idx_s = pp.tile([P, W * 4 * 32], i16)
            nc.sync.dma_start(out=idx_s[:], in_=IDX[:, :])
            rowid_s = pp.tile([P, W * 16], b16)
            nc.sync.dma_start(out=rowid_s[:], in_=ROWID[:, :])
            rdeg_s = pp.tile([P, W], f32)
            nc.sync.dma_start(out=rdeg_s[:], in_=RDEG[:, :])

            wl_s = [wp.tile([C, C], b16, tag=f"wl{l}", name=f"wl{l}") for l in range(2)] + [wp.tile([C, OUT], b16, tag="wl2_", name="wl2_")]
            wr_s = [wp.tile([C, C], b16, tag=f"wr{l}", name=f"wr{l}") for l in range(2)] + [wp.tile([C, OUT], b16, tag="wr2_", name="wr2_")]
            bv_s = [wp.tile([1, C], b16, tag=f"bv{l}", name=f"bv{l}") for l in range(2)] + [wp.tile([1, OUT], b16, tag="bv2_", name="bv2_")]
            for l in range(3):
                nc.sync.dma_start(out=wl_s[l][:], in_=WL[l][:, :])
                nc.sync.dma_start(out=wr_s[l][:], in_=WR[l][:, :])
                nc.sync.dma_start(out=bv_s[l][:], in_=BV[l][:, :])
            we1a_s = wp.tile([OUT, C], b16); nc.sync.dma_start(out=we1a_s[:], in_=WE1A[:, :])
            we1b_s = wp.tile([OUT, C], b16); nc.sync.dma_start(out=we1b_s[:], in_=WE1B[:, :])
            be1_s = wp.tile([C, 1], f32); nc.sync.dma_start(out=be1_s[:], in_=BE1[:, :])
            we2_s = wp.tile([C, 1], b16); nc.sync.dma_start(out=we2_s[:], in_=WE2[:, :])
            be2_s = wp.tile([P, 1], f32)
            nc.vector.memset(be2_s[:], BE2)

            hT = [pp.tile([P, W * P], b16, tag=f"hT{l}", name=f"hT{l}") for l in range(2)]
            hres = [pp.tile([P, W * P], b16, tag=f"hres{l}", name=f"hres{l}") for l in range(2)]

            didxs_s = pp.tile([P, IDXCOLS_D], i16)
            nc.sync.dma_start(out=didxs_s[:], in_=DIDXS[:, :])
            didxd_s = pp.tile([P, IDXCOLS_D], i16)
            nc.sync.dma_start(out=didxd_s[:], in_=DIDXD[:, :])

            # ---------------- conv layers ----------------
            for l in range(3):
                CO = C if l < 2 else OUT
                with (
                    tc.tile_pool(name=f"msg{l}", bufs=3) as mp,
                    tc.tile_pool(name=f"s01p{l}", bufs=3) as sp,
                    tc.tile_pool(name=f"ep{l}", bufs=2) as ep,
                    tc.tile_pool(name=f"ps{l}", bufs=2, space="PSUM") as psp,
                ):
                    for w in range(W):
                        msg = mp.tile([P, 16, C], b16, tag="msg")
                        if l == 0:
                            nc.sync.dma_start(
                                out=msg[:], in_=XSLOTS[:, w * 16 * C:(w + 1) * 16 * C])
                        else:
                            for q in range(4):
                                nc.gpsimd.dma_gather(
                                    out_ap=msg[:, q * 4:(q + 1) * 4, :],
                                    in_ap=htab[l - 1][q * QR:(q + 1) * QR, :],
                                    idxs_ap=idx_s[:, (w * 4 + q) * 32:(w * 4 + q + 1) * 32],
                                    num_idxs=QSLOTS, num_idxs_reg=QSLOTS,
                                    elem_size=C, queue_num=next_q())
                        ps_agg = psp.tile([C, P], f32, tag="agg")
                        for j4 in range(4):
                            s01 = sp.tile([P, 4, P], b16, tag="s01")
                            nc.vector.tensor_tensor(
                                out=s01[:], in0=iota[:],
                                in1=rowid_s[:, w * 16 + j4 * 4: w * 16 + (j4 + 1) * 4].to_broadcast([P, 4, P]),
                                op=mybir.AluOpType.is_equal)
                            for jj in range(4):
                                j = j4 * 4 + jj
                                nc.tensor.matmul(
                                    ps_agg[:], lhsT=msg[:, j, :], rhs=s01[:, jj, :],
                                    start=(j == 0), stop=(j == 15))
                        aggTs = ep.tile([C, P], b16, tag="aggTs")
                        nc.scalar.copy(out=aggTs[:], in_=ps_agg[:])
                        ps_y1 = psp.tile([P, CO], f32, tag="y1")
                        nc.tensor.matmul(ps_y1[:], lhsT=aggTs[:], rhs=wl_s[l][:], start=True, stop=True)
                        ps_y2 = psp.tile([P, CO], f32, tag="y2")
                        root = XT[:, w * P:(w + 1) * P] if l == 0 else hT[l - 1][:, w * P:(w + 1) * P]
                        if l == 0:
                            rootT = ep.tile([P, P], b16, tag="rootT")
                            nc.sync.dma_start(out=rootT[:], in_=root)
                            root = rootT[:]
                        nc.tensor.matmul(ps_y2[:], lhsT=root, rhs=wr_s[l][:], start=True, stop=False)
                        nc.tensor.matmul(ps_y2[:], lhsT=ones1[:], rhs=bv_s[l][:], start=False, stop=True)
                        ymean = ep.tile([P, CO], f32, tag="ymean")
                        nc.vector.tensor_scalar_mul(ymean[:], ps_y1[:], rdeg_s[:, w:w + 1])
                        y = ep.tile([P, CO], f32, tag="y")
                        nc.vector.tensor_add(y[:], ymean[:], ps_y2[:])
                        if l < 2:
                            hrel = ep.tile([P, CO], b16, tag="hrel")
                            nc.scalar.activation(out=hrel[:], in_=y[:], func=AF.Relu)
                            resin = XRES[:, w * P:(w + 1) * P] if l == 0 else hres[l - 1][:, w * P:(w + 1) * P]
                            if l == 0:
                                resT = ep.tile([P, P], b16, tag="resT")
                                nc.sync.dma_start(out=resT[:], in_=resin)
                                resin = resT[:]
                            nc.vector.tensor_add(hres[l][:, w * P:(w + 1) * P], hrel[:], resin)
                            ps_t = psp.tile([P, P], b16, tag="tp")
                            nc.tensor.transpose(ps_t[:], hres[l][:, w * P:(w + 1) * P], ident[:])
                            nc.scalar.copy(out=hT[l][:, w * P:(w + 1) * P], in_=ps_t[:])
                            nc.sync.dma_start(out=bounce[l][w * P:(w + 1) * P, :],
                                              in_=hres[l][:, w * P:(w + 1) * P])
                        else:
                            nc.sync.dma_start(out=Z[w * P:(w + 1) * P, :], in_=y[:])
                            nc.sync.dma_start(out=bounce[l][w * P:(w + 1) * P, :], in_=y[:])
                nc.gpsimd.collective_compute(
                        "AllGather", mybir.AluOpType.bypass,
                        replica_groups=[list(range(NCORES))],
                        ins=[bounce[l][:, :].opt()], outs=[htab[l][:, :].opt()])

            # ---------------- decode ----------------
            with (
                tc.tile_pool(name="dmp", bufs=3) as dmp,
                tc.tile_pool(name="dep", bufs=3) as dep,
                tc.tile_pool(name="dps", bufs=2, space="PSUM") as dps,
            ):
                dstage = pp.tile([P, TD], f32)
                gstart = {}
                gtile_count = {g: 0 for g in range(16)}
                for g, k in call_plan:
                    gtile_count[g] += k
                acc = 0
                for g in range(16):
                    gstart[g] = acc
                    acc += gtile_count[g]
                tcur = {g: 0 for g in range(16)}
                col = 0
                for (g, k) in call_plan:
                    qs, qd = g // 4, g % 4
                    zs = dmp.tile([P, 8, OUT], f32, tag="zs")
                    zd = dmp.tile([P, 8, OUT], f32, tag="zd")
                    nc.gpsimd.dma_gather(
                        out_ap=zs[:, :k, :], in_ap=htab[2][qs * QR:(qs + 1) * QR, :],
                        idxs_ap=didxs_s[:, col:col + k * 8],
                        num_idxs=k * 128, num_idxs_reg=k * 128,
                        elem_size=OUT, queue_num=next_q())
                    nc.gpsimd.dma_gather(
                        out_ap=zd[:, :k, :], in_ap=htab[2][qd * QR:(qd + 1) * QR, :],
                        idxs_ap=didxd_s[:, col:col + k * 8],
                        num_idxs=k * 128, num_idxs_reg=k * 128,
                        elem_size=OUT, queue_num=next_q())
                    for j in range(k):
                        gt_col = gstart[g] + tcur[g] + j
                        ps_zs = dps.tile([OUT, P], f32, tag="pzs")
                        nc.tensor.transpose(ps_zs[:], zs[:, j, :], identf[:])
                        zsT = dep.tile([OUT, P], b16, tag="zsT")
                        nc.scalar.copy(out=zsT[:], in_=ps_zs[:])
                        ps_zd = dps.tile([OUT, P], f32, tag="pzd")
                        nc.tensor.transpose(ps_zd[:], zd[:, j, :], identf[:])
                        zdT = dep.tile([OUT, P], b16, tag="zdT")
                        nc.scalar.copy(out=zdT[:], in_=ps_zd[:])
                        ps_h = dps.tile([C, P], f32, tag="ph")
                        nc.tensor.matmul(ps_h[:], lhsT=we1a_s[:], rhs=zsT[:], start=True, stop=False)
                        nc.tensor.matmul(ps_h[:], lhsT=we1b_s[:], rhs=zdT[:], start=False, stop=True)
                        hidT = dep.tile([C, P], b16, tag="hidT")
                        nc.scalar.activation(out=hidT[:], in_=ps_h[:], func=AF.Relu, bias=be1_s[:])
                        ps_o = dps.tile([P, 1], f32, tag="po")
                        nc.tensor.matmul(ps_o[:], lhsT=hidT[:], rhs=we2_s[:], start=True, stop=True)
                        nc.scalar.activation(out=dstage[:, gt_col:gt_col + 1], in_=ps_o[:],
                                             func=AF.Sigmoid, bias=be2_s[:])
                    tcur[g] += k
                    col += k * 8
                nc.sync.dma_start(out=DOUT[:, :], in_=dstage[:])

    nc.compile()
    return nc


# ---------------------------------------------------------------- run

def kernel(**inputs):
    import time
    global LAST_EXEC_NS
    _install_ntff_shim()
    from concourse import bass_utils

    t0 = time.time()
    per_core, wts, meta = preprocess(inputs)
    print(f"[kernel] preprocess {time.time()-t0:.1f}s W={meta['W']} TD={meta['TD']}", flush=True)
    meta["BE2"] = wts["BE2"]
    t1 = time.time()
    nc = build_program(meta)
    print(f"[kernel] build+compile {time.time()-t1:.1f}s", flush=True)

    in_maps = []
    for c in range(NCORES):
        d = per_core[c]
        m = {k: d[k] for k in ("IDX", "ROWID", "RDEG", "XT", "XRES", "XSLOTS", "DIDXS", "DIDXD")}
        for k in ("WL1", "WR1", "B1", "WL2", "WR2", "B2", "WL3", "WR3", "B3",
                  "WE1A", "WE1B", "BE1", "WE2"):
            m[k] = wts[k]
        in_maps.append(m)

    trace = bool(os.environ.get("GS_TRACE"))
    t2 = time.time()
    res = bass_utils.run_bass_kernel_spmd(
        nc, in_maps, core_ids=list(range(NCORES)), trace=trace)
    LAST_EXEC_NS = res.exec_time_ns
    print(f"[kernel] run {time.time()-t2:.1f}s exec_ns={LAST_EXEC_NS}", flush=True)

    # ---------------- unshard ----------------
    N, OUT, W, TD = meta["N"], meta["OUT"], meta["W"], meta["TD"]
    z = np.zeros((N, OUT), np.float32)
    for c in range(NCORES):
        nodes = per_core[c]["nodes"]            # [W, 128]
        Zc = res.results[c]["Z"].reshape(W, P, OUT)
        valid = nodes >= 0
        z[nodes[valid]] = Zc[valid]
    ND, DSH = meta["ND"], meta["DSH"]
    douts = np.zeros(ND, np.float32)
    for c in range(NCORES):
        sm = per_core[c]["slotmap"]             # [TD, 128]
        do = res.results[c]["DOUT"]             # [128, TD]
        valid = sm >= 0
        douts[sm[valid]] = do.T[valid]
    pos = douts[:ND // 2]
    neg = douts[ND // 2:]
    return z, pos, neg

